# revision 33
# baseline (speedup 1.0000x reference)
"""CrossModalAttention Trainium2 kernel.

Full inputs -> full output. Internally: 8-way SPMD over (batch, key-half):
core = 2*b + h owns keys [h*2048, (h+1)*2048) of batch b and computes the
UNNORMALIZED attention output over those keys for ALL 4096 queries, plus
the per-query partition sum Z. The host sums the two partials per batch
and normalizes.

Math (per batch), with x = concat(img, label, z) [C=256, N=4096]:
  q = wq x + bq, k = wk x (bk dropped: constant-in-key terms cancel in
  softmax), v = wv x + bv
  S[n, m] = q_n . k_m = x_n^T A x_m + t_m       A = wq^T wk,  t = (wk^T bq)^T x
All three projections (kk = A x on the key side, t, vT) are computed on
the HOST: the device runs only the two N^2 C stages (scores + PV), in
fp16 operands (fp32 PSUM accumulation) so LDWEIGHTS gets fast-weight-load
and the input DMA bytes halve vs fp32.

Layouts (all per core, m = this core's 2048 keys):
  ST[m, n] via lhsT = kk chunk [128c, 128m], rhs = xq [128c, 512n]
  P = exp(ST + (t[m] - SHIFT))  -> bf16 pt
  out[n, c] = sum_m pt[m, n] * vT[m, c]; vT has ones columns appended so
  the same accumulation yields Z[n]. Raw (out|Z) goes to HBM; the host
  sums the two key-halves, divides by Z and adds bv.

Schedule notes:
- ~6 warmup matmuls on a zeroed fp16 tile keep the PE busy from the end
  of the framework preamble until the first kk/xq pieces land, so the
  HAM clock-gate sees one continuous busy window (2.4 GHz by ~10.7 us).
- Inputs ride four DMA queues (sync/gpsimd: kk halves + vT tail,
  scalar: xq, vector: vT head), pieces ordered by first-use.
- Score PSUM tiles are [128,512] (1 bank) with bufs=6: the exp (ACT) is
  slightly slower per tile than the 2 matmuls that fill it, and a deep
  rotation absorbs the drift without stalling the PE.
- Block interleave [ST0][ST1][PV0][ST2][PV1]...: PV(nb) starts a full
  score block after ST(nb), so exp(nb) is always done; pt is
  double-buffered. The last PV block interleaves its first two
  sub-blocks to cover the final exp tail.
- SHIFT=85 as in the proven baseline: scores lie in [-128, 132], exp in
  fp32 range with margin. Partial sums stay finite in fp32 (Z <= 2048*e^47).
"""

import numpy as np

import concourse.bacc as bacc
import concourse.mybir as mybir
import concourse.tile as tile
from concourse import bass_utils

B = 4
C = 256  # channels after concat
H = W = 64
N = H * W  # 4096 pixels
NCORES = 8
MHALF = N // 2  # 2048 keys per core
SHIFT = 85.0

F32 = mybir.dt.float32
F16 = mybir.dt.float16
BF16 = mybir.dt.bfloat16

FQ = 512  # query-block free dim
NB = N // FQ  # 8 query blocks per core (all queries)
MJ = MHALF // 128  # 16 key chunks of 128
CA = C + 2  # channels + ones col + pad (even free dim for the PE)


def _emit(nc, tc, kk_d, xq_d, vt_d, tb_d, out_d):
    f32 = F32
    mm = nc.tensor.matmul
    Exp = mybir.ActivationFunctionType.Exp
    Copy = mybir.ActivationFunctionType.Copy

    with tc.tile_pool(name="consts", bufs=1) as cp, \
         tc.tile_pool(name="xp", bufs=1) as xp, \
         tc.tile_pool(name="bigps", bufs=6, space="PSUM") as bigps, \
         tc.tile_pool(name="attn", bufs=2) as app, \
         tc.tile_pool(name="ob", bufs=6) as op, \
         tc.tile_pool(name="vps", bufs=2, space="PSUM") as vps:
        tb = cp.tile([128, MJ], f32, name="tb", tag="tb")
        warm = cp.tile([128, 512], F16, name="warm", tag="warm")
        # split memset: the first warmup LDWEIGHTS only needs cols 0:128
        nc.vector.memset(warm[:, 0:128], 0.0)
        nc.vector.memset(warm[:, 128:512], 0.0)

        kk = [xp.tile([128, MHALF], F16, name=f"kk{i}", tag=f"kk{i}")
              for i in range(2)]
        xq = [xp.tile([128, N], F16, name=f"xq{i}", tag=f"xq{i}")
              for i in range(2)]
        vT = xp.tile([128, MJ * CA], BF16, name="vT", tag="vT")

        # ---- DMAs: three queues (only sync/gpsimd/scalar can kick),
        # pieces ordered by first use. xq is host-rotated so this
        # core's 2048 keys are columns 0:2048 — the leading xq pieces
        # feed BOTH the kk projection (keys) and ST0-3 (queries). The
        # kk cascade consumes cols ascending, so xq streams in order.
        # AT (tiny) + tb lead on scalar; vT (needed from PV0, ~25 us)
        # rides scalar after them, keeping sync/gpsimd free for output
        # pieces from ~25 us on.
        nc.sync.dma_start(xq[0][:, 0:512], xq_d.ap()[0:128, 0:512])
        nc.sync.dma_start(kk[0][:, 0:256], kk_d.ap()[0:128, 0:256])
        nc.sync.dma_start(kk[0][:, 256:1024], kk_d.ap()[0:128, 256:1024])
        nc.sync.dma_start(xq[0][:, 512:1024], xq_d.ap()[0:128, 512:1024])
        nc.sync.dma_start(vT[:, 6 * CA:11 * CA], vt_d.ap()[:, 6 * CA:11 * CA])
        nc.sync.dma_start(xq[0][:, 1024:2048], xq_d.ap()[0:128, 1024:2048])
        nc.sync.dma_start(xq[0][:, 2048:4096], xq_d.ap()[0:128, 2048:4096])
        nc.gpsimd.dma_start(xq[1][:, 0:512], xq_d.ap()[128:256, 0:512])
        nc.gpsimd.dma_start(kk[1][:, 0:256], kk_d.ap()[128:256, 0:256])
        nc.gpsimd.dma_start(kk[1][:, 256:1024], kk_d.ap()[128:256, 256:1024])
        nc.gpsimd.dma_start(xq[1][:, 512:1024], xq_d.ap()[128:256, 512:1024])
        nc.gpsimd.dma_start(vT[:, 11 * CA:16 * CA],
                            vt_d.ap()[:, 11 * CA:16 * CA])
        nc.gpsimd.dma_start(xq[1][:, 1024:2048], xq_d.ap()[128:256, 1024:2048])
        nc.gpsimd.dma_start(xq[1][:, 2048:4096],
                            xq_d.ap()[128:256, 2048:4096])
        nc.scalar.dma_start(tb[:], tb_d.ap()[:, :])
        nc.scalar.dma_start(kk[0][:, 1024:2048], kk_d.ap()[0:128, 1024:2048])
        nc.scalar.dma_start(kk[1][:, 1024:2048], kk_d.ap()[128:256, 1024:2048])
        nc.scalar.dma_start(vT[:, 0:6 * CA], vt_d.ap()[:, 0:6 * CA])

        # ---- PE warmup (HAM un-throttle): keep the PE busy from the
        # end of the framework preamble until the first input pieces
        # land, so the clock-gate sees one continuous busy window.
        wps = bigps.tile([128, 512], f32, name="wps", tag="ps")

        def warm_mms(n):
            for _ in range(n):
                mm(wps[:], warm[:, 0:128], warm[:], start=True, stop=True)

        # Warmup bridges from the end of the framework preamble until
        # the DMA stream catches up (~12 us): one continuous PE-busy
        # window so the HAM clock-gate reaches 2.4 GHz at ~10.7 us
        # (which also doubles the DMA rate).
        warm_mms(9)

        # ---- attention ----
        def st_range(nb, ptb, mja, mjb):
            for mj in range(mja, mjb):
                ps = bigps.tile([128, 512], f32, name="st", tag="ps")
                for ci in range(2):
                    mm(ps[:], kk[ci][:, mj * 128:(mj + 1) * 128],
                       xq[ci][:, nb * FQ:(nb + 1) * FQ],
                       start=ci == 0, stop=ci == 1)
                nc.scalar.activation(
                    ptb[:, mj * FQ:(mj + 1) * FQ], ps[:], Exp,
                    bias=tb[:, mj:mj + 1])

        def pv_mm(po, ptb, ns, mj, start, stop):
            o = mj * FQ + ns * 128
            mm(po[:], ptb[:, o:o + 128], vT[:, mj * CA:(mj + 1) * CA],
               start=start, stop=stop)

        # Output rides as bf16 (the DMA-ring write path is the scarce
        # resource). The fp32 Z is bit-split across the last two bf16
        # columns (exact — the host reassembles the fp32 bits).
        def pv_finish(po, nb, ns, eng=None):
            ob = op.tile([128, CA], BF16, name="ob", tag="ob")
            nc.vector.tensor_copy(ob[:, 0:C], po[:, 0:C])
            nc.vector.tensor_copy(ob[:, C:C + 2].bitcast(F32),
                                  po[:, C:C + 1])
            r = nb * FQ + ns * 128
            if eng is None:
                # scalar's input ring drains by ~16 us (sync/gpsimd
                # still carry the xq tails) — give it half the output.
                eng = (nc.scalar, nc.gpsimd, nc.scalar, nc.sync)[ns]
            eng.dma_start(out_d.ap()[r:r + 128, :], ob[:])

        def pv_block(nb, ptb, last):
            if not last:
                for ns in range(4):
                    po = vps.tile([128, CA], f32, name="pv", tag="pv")
                    for mj in range(MJ):
                        pv_mm(po, ptb, ns, mj, mj == 0, mj == MJ - 1)
                    pv_finish(po, nb, ns)
                return
            # last block: interleave the first two sub-blocks so the
            # accumulation never waits on the trailing exp chunks.
            po0 = vps.tile([128, CA], f32, name="pv0", tag="pv")
            for mj in range(12):
                pv_mm(po0, ptb, 0, mj, mj == 0, False)
            po1 = vps.tile([128, CA], f32, name="pv1", tag="pv")
            for mj in range(8):
                pv_mm(po1, ptb, 1, mj, mj == 0, False)
            # final-block tail: all DMAs ride the scalar queue (its ring
            # is empty by now — the gpsimd/sync rings still carry earlier
            # blocks), and the last sub-block's Z hi/lo chain runs on
            # scalar in parallel with the bulk copy on vector.
            for mj in range(12, MJ):
                pv_mm(po0, ptb, 0, mj, False, mj == MJ - 1)
            pv_finish(po0, nb, 0, nc.scalar)
            for mj in range(8, MJ):
                pv_mm(po1, ptb, 1, mj, False, mj == MJ - 1)
            pv_finish(po1, nb, 1, nc.scalar)
            po2 = vps.tile([128, CA], f32, name="pv", tag="pv")
            for mj in range(MJ):
                pv_mm(po2, ptb, 2, mj, mj == 0, mj == MJ - 1)
            pv_finish(po2, nb, 2, nc.scalar)
            po3 = vps.tile([128, CA], f32, name="pv", tag="pv")
            for mj in range(MJ):
                pv_mm(po3, ptb, 3, mj, mj == 0, mj == MJ - 1)
            ob = op.tile([128, CA], BF16, name="ob", tag="ob")
            nc.scalar.activation(ob[:, 0:C], po3[:, 0:C], Copy)
            nc.vector.tensor_copy(ob[:, C:C + 2].bitcast(F32),
                                  po3[:, C:C + 1])
            r = nb * FQ + 3 * 128
            nc.scalar.dma_start(out_d.ap()[r:r + 64, :], ob[0:64, :])
            nc.sync.dma_start(out_d.ap()[r + 64:r + 128, :], ob[64:128, :])

        # phase0/ST0 cascade, piece-major: each xq piece feeds its kk
        # segment, which immediately unblocks the matching ST0 chunks.
        pt0 = app.tile([128, MJ * FQ], BF16, name="pt", tag="pt")
        # warm-fill inside ST0 bridges the kk mid-piece arrivals (the
        # PE must stay gap-free or the HAM re-throttles to 1.2 GHz,
        # which also halves the DMA rate).
        st_range(0, pt0, 0, 2)
        warm_mms(2)
        st_range(0, pt0, 2, 4)
        warm_mms(2)
        st_range(0, pt0, 4, 6)
        warm_mms(1)
        st_range(0, pt0, 6, MJ)
        pts = [pt0]
        for nb in range(1, NB):
            ptb = app.tile([128, MJ * FQ], BF16, name="pt", tag="pt")
            pts.append(ptb)
            st_range(nb, ptb, 0, MJ)
            pv_block(nb - 1, pts[nb - 1], False)
        pv_block(NB - 1, pts[NB - 1], True)


_CACHE = {}


def _build():
    if "nc" in _CACHE:
        return _CACHE["nc"]
    nc = bacc.Bacc("TRN2", target_bir_lowering=False, debug=False)
    kk_d = nc.dram_tensor("kk", [C, MHALF], F16, kind="ExternalInput")
    xq_d = nc.dram_tensor("xq", [C, N], F16, kind="ExternalInput")
    vt_d = nc.dram_tensor("vt", [128, MJ * CA], BF16, kind="ExternalInput")
    tb_d = nc.dram_tensor("tb", [128, MJ], F32, kind="ExternalInput")
    out_d = nc.dram_tensor("out", [N, CA], BF16, kind="ExternalOutput")
    with tile.TileContext(nc) as tc:
        _emit(nc, tc, kk_d, xq_d, vt_d, tb_d, out_d)
    nc.compile()
    _CACHE["nc"] = nc
    return nc


def _in_maps(img, label, z, wq, bq, wk, bk, wv, bv):
    bf16 = mybir.dt.np(BF16)
    x = np.concatenate(
        [np.asarray(img), np.asarray(label), np.asarray(z)], axis=1
    ).reshape(B, C, N).astype(np.float32)
    wq64 = np.asarray(wq, np.float64)
    wk64 = np.asarray(wk, np.float64)
    A = (wq64.T @ wk64).astype(np.float32)  # S = x^T A x + t
    u = (wk64.T @ np.asarray(bq, np.float64)).astype(np.float32)
    wvf = np.asarray(wv, np.float32)
    maps = []
    for b in range(B):
        kkb = A @ x[b]                      # [C, N]
        tbf = u @ x[b]                      # [N]
        vb = wvf @ x[b]                     # [C, N] (bv added on host)
        xq16 = np.ascontiguousarray(x[b].astype(np.float16))
        for h in range(2):
            sl = slice(h * MHALF, (h + 1) * MHALF)
            kk16 = np.ascontiguousarray(kkb[:, sl].astype(np.float16))
            tbv = np.ascontiguousarray(
                tbf[sl].reshape(MJ, 128).T.astype(np.float32)
            ) - np.float32(SHIFT)
            vt = np.ones((128, MJ, CA), np.float32)
            vt[:, :, 0:C] = vb[:, sl].reshape(C, MJ, 128).transpose(2, 1, 0)
            vt16 = np.ascontiguousarray(
                vt.reshape(128, MJ * CA).astype(bf16))
            maps.append({"kk": kk16, "xq": xq16, "vt": vt16, "tb": tbv})
    return maps


def kernel(img, label, z, wq, bq, wk, bk, wv, bv):
    nc = _build()
    maps = _in_maps(img, label, z, wq, bq, wk, bk, wv, bv)
    res = bass_utils.run_bass_kernel_spmd(nc, maps,
                                          core_ids=list(range(NCORES)))

    def _split(raw):
        # vals in bf16; Z arrives as raw fp32 bits spread over the last
        # two bf16 columns
        u = np.ascontiguousarray(raw).view(np.uint16)
        zz = (u[:, C].astype(np.uint32)
              | (u[:, C + 1].astype(np.uint32) << 16)).view(np.float32)
        return raw[:, 0:C].astype(np.float32), zz.reshape(-1, 1)

    out = np.empty((B, C, N), np.float32)
    bvf = np.asarray(bv, np.float32).reshape(1, C)
    for b in range(B):
        o0, z0 = _split(res.results[2 * b]["out"])
        o1, z1 = _split(res.results[2 * b + 1]["out"])
        o = o0 + o1
        zz = z0 + z1
        out[b] = ((o / zz) + bvf).T
    return out.reshape(B, C, H, W)


# revision 35
# speedup vs baseline: 1.0155x; 1.0155x over previous
"""CrossModalAttention Trainium2 kernel.

Full inputs -> full output. Internally: 8-way SPMD over (batch, key-half):
core = 2*b + h owns keys [h*2048, (h+1)*2048) of batch b and computes the
UNNORMALIZED attention output over those keys for ALL 4096 queries, plus
the per-query partition sum Z. The host sums the two partials per batch
and normalizes.

Math (per batch), with x = concat(img, label, z) [C=256, N=4096]:
  q = wq x + bq, k = wk x (bk dropped: constant-in-key terms cancel in
  softmax), v = wv x + bv
  S[n, m] = q_n . k_m = x_n^T A x_m + t_m       A = wq^T wk,  t = (wk^T bq)^T x
All three projections (kk = A x on the key side, t, vT) are computed on
the HOST: the device runs only the two N^2 C stages (scores + PV), in
fp16 operands (fp32 PSUM accumulation) so LDWEIGHTS gets fast-weight-load
and the input DMA bytes halve vs fp32.

Layouts (all per core, m = this core's 2048 keys):
  ST[m, n] via lhsT = kk chunk [128c, 128m], rhs = xq [128c, 512n]
  P = exp(ST + (t[m] - SHIFT))  -> bf16 pt
  out[n, c] = sum_m pt[m, n] * vT[m, c]; vT has ones columns appended so
  the same accumulation yields Z[n]. Raw (out|Z) goes to HBM; the host
  sums the two key-halves, divides by Z and adds bv.

Schedule notes:
- ~6 warmup matmuls on a zeroed fp16 tile keep the PE busy from the end
  of the framework preamble until the first kk/xq pieces land, so the
  HAM clock-gate sees one continuous busy window (2.4 GHz by ~10.7 us).
- Inputs ride four DMA queues (sync/gpsimd: kk halves + vT tail,
  scalar: xq, vector: vT head), pieces ordered by first-use.
- Score PSUM tiles are [128,512] (1 bank) with bufs=6: the exp (ACT) is
  slightly slower per tile than the 2 matmuls that fill it, and a deep
  rotation absorbs the drift without stalling the PE.
- Block interleave [ST0][ST1][PV0][ST2][PV1]...: PV(nb) starts a full
  score block after ST(nb), so exp(nb) is always done; pt is
  double-buffered. The last PV block interleaves its first two
  sub-blocks to cover the final exp tail.
- SHIFT=85 as in the proven baseline: scores lie in [-128, 132], exp in
  fp32 range with margin. Partial sums stay finite in fp32 (Z <= 2048*e^47).
"""

import numpy as np

import concourse.bacc as bacc
import concourse.mybir as mybir
import concourse.tile as tile
from concourse import bass_utils

B = 4
C = 256  # channels after concat
H = W = 64
N = H * W  # 4096 pixels
NCORES = 8
MHALF = N // 2  # 2048 keys per core
SHIFT = 85.0

F32 = mybir.dt.float32
F16 = mybir.dt.float16
BF16 = mybir.dt.bfloat16

FQ = 512  # query-block free dim
NB = N // FQ  # 8 query blocks per core (all queries)
MJ = MHALF // 128  # 16 key chunks of 128
CA = C + 2  # channels + ones col + pad (even free dim for the PE)


def _emit(nc, tc, kk_d, xq_d, vt_d, tb_d, out_d):
    f32 = F32
    mm = nc.tensor.matmul
    Exp = mybir.ActivationFunctionType.Exp
    Copy = mybir.ActivationFunctionType.Copy

    with tc.tile_pool(name="consts", bufs=1) as cp, \
         tc.tile_pool(name="xp", bufs=1) as xp, \
         tc.tile_pool(name="bigps", bufs=6, space="PSUM") as bigps, \
         tc.tile_pool(name="attn", bufs=2) as app, \
         tc.tile_pool(name="ob", bufs=6) as op, \
         tc.tile_pool(name="vps", bufs=2, space="PSUM") as vps:
        tb = cp.tile([128, MJ], f32, name="tb", tag="tb")
        warm = cp.tile([128, 512], F16, name="warm", tag="warm")
        # split memset: the first warmup LDWEIGHTS only needs cols 0:128
        nc.vector.memset(warm[:, 0:128], 0.0)
        nc.vector.memset(warm[:, 128:512], 0.0)

        kk = [xp.tile([128, MHALF], F16, name=f"kk{i}", tag=f"kk{i}")
              for i in range(2)]
        xq = [xp.tile([128, N], F16, name=f"xq{i}", tag=f"xq{i}")
              for i in range(2)]
        vT = xp.tile([128, MJ * CA], BF16, name="vT", tag="vT")

        # ---- DMAs: three queues (only sync/gpsimd/scalar can kick),
        # pieces ordered by first use. xq is host-rotated so this
        # core's 2048 keys are columns 0:2048 — the leading xq pieces
        # feed BOTH the kk projection (keys) and ST0-3 (queries). The
        # kk cascade consumes cols ascending, so xq streams in order.
        # AT (tiny) + tb lead on scalar; vT (needed from PV0, ~25 us)
        # rides scalar after them, keeping sync/gpsimd free for output
        # pieces from ~25 us on.
        nc.sync.dma_start(xq[0][:, 0:512], xq_d.ap()[0:128, 0:512])
        nc.sync.dma_start(kk[0][:, 0:256], kk_d.ap()[0:128, 0:256])
        nc.sync.dma_start(kk[0][:, 256:1024], kk_d.ap()[0:128, 256:1024])
        nc.sync.dma_start(xq[0][:, 512:1024], xq_d.ap()[0:128, 512:1024])
        nc.sync.dma_start(vT[:, 6 * CA:11 * CA], vt_d.ap()[:, 6 * CA:11 * CA])
        nc.sync.dma_start(xq[0][:, 1024:2048], xq_d.ap()[0:128, 1024:2048])
        nc.sync.dma_start(xq[0][:, 2048:4096], xq_d.ap()[0:128, 2048:4096])
        nc.gpsimd.dma_start(xq[1][:, 0:512], xq_d.ap()[128:256, 0:512])
        nc.gpsimd.dma_start(kk[1][:, 0:256], kk_d.ap()[128:256, 0:256])
        nc.gpsimd.dma_start(kk[1][:, 256:1024], kk_d.ap()[128:256, 256:1024])
        nc.gpsimd.dma_start(xq[1][:, 512:1024], xq_d.ap()[128:256, 512:1024])
        nc.gpsimd.dma_start(vT[:, 11 * CA:16 * CA],
                            vt_d.ap()[:, 11 * CA:16 * CA])
        nc.gpsimd.dma_start(xq[1][:, 1024:2048], xq_d.ap()[128:256, 1024:2048])
        nc.gpsimd.dma_start(xq[1][:, 2048:4096],
                            xq_d.ap()[128:256, 2048:4096])
        nc.scalar.dma_start(tb[:], tb_d.ap()[:, :])
        nc.scalar.dma_start(kk[0][:, 1024:2048], kk_d.ap()[0:128, 1024:2048])
        nc.scalar.dma_start(kk[1][:, 1024:2048], kk_d.ap()[128:256, 1024:2048])
        nc.scalar.dma_start(vT[:, 0:6 * CA], vt_d.ap()[:, 0:6 * CA])

        # ---- PE warmup (HAM un-throttle): keep the PE busy from the
        # end of the framework preamble until the first input pieces
        # land, so the clock-gate sees one continuous busy window.
        wps = bigps.tile([128, 512], f32, name="wps", tag="ps")

        def warm_mms(n):
            for _ in range(n):
                mm(wps[:], warm[:, 0:128], warm[:], start=True, stop=True)

        # Warmup bridges from the end of the framework preamble until
        # the DMA stream catches up (~12 us): one continuous PE-busy
        # window so the HAM clock-gate reaches 2.4 GHz at ~10.7 us
        # (which also doubles the DMA rate).
        warm_mms(10)

        # ---- attention ----
        def st_range(nb, ptb, mja, mjb):
            for mj in range(mja, mjb):
                ps = bigps.tile([128, 512], f32, name="st", tag="ps")
                for ci in range(2):
                    mm(ps[:], kk[ci][:, mj * 128:(mj + 1) * 128],
                       xq[ci][:, nb * FQ:(nb + 1) * FQ],
                       start=ci == 0, stop=ci == 1)
                nc.scalar.activation(
                    ptb[:, mj * FQ:(mj + 1) * FQ], ps[:], Exp,
                    bias=tb[:, mj:mj + 1])

        def pv_mm(po, ptb, ns, mj, start, stop):
            o = mj * FQ + ns * 128
            mm(po[:], ptb[:, o:o + 128], vT[:, mj * CA:(mj + 1) * CA],
               start=start, stop=stop)

        # Output rides as bf16 (the DMA-ring write path is the scarce
        # resource). The fp32 Z is bit-split across the last two bf16
        # columns (exact — the host reassembles the fp32 bits).
        def pv_finish(po, nb, ns, eng=None):
            ob = op.tile([128, CA], BF16, name="ob", tag="ob")
            nc.vector.tensor_copy(ob[:, 0:C], po[:, 0:C])
            nc.vector.tensor_copy(ob[:, C:C + 2].bitcast(F32),
                                  po[:, C:C + 1])
            r = nb * FQ + ns * 128
            if eng is None:
                # scalar's input ring drains by ~16 us (sync/gpsimd
                # still carry the xq tails) — give it half the output.
                eng = (nc.scalar, nc.gpsimd, nc.scalar, nc.sync)[ns]
            eng.dma_start(out_d.ap()[r:r + 128, :], ob[:])

        def pv_block(nb, ptb, last):
            if not last:
                for ns in range(4):
                    po = vps.tile([128, CA], f32, name="pv", tag="pv")
                    for mj in range(MJ):
                        pv_mm(po, ptb, ns, mj, mj == 0, mj == MJ - 1)
                    pv_finish(po, nb, ns)
                return
            # last block: interleave the first two sub-blocks so the
            # accumulation never waits on the trailing exp chunks.
            po0 = vps.tile([128, CA], f32, name="pv0", tag="pv")
            for mj in range(12):
                pv_mm(po0, ptb, 0, mj, mj == 0, False)
            po1 = vps.tile([128, CA], f32, name="pv1", tag="pv")
            for mj in range(8):
                pv_mm(po1, ptb, 1, mj, mj == 0, False)
            # final-block tail: all DMAs ride the scalar queue (its ring
            # is empty by now — the gpsimd/sync rings still carry earlier
            # blocks), and the last sub-block's Z hi/lo chain runs on
            # scalar in parallel with the bulk copy on vector.
            for mj in range(12, MJ):
                pv_mm(po0, ptb, 0, mj, False, mj == MJ - 1)
            pv_finish(po0, nb, 0, nc.scalar)
            for mj in range(8, MJ):
                pv_mm(po1, ptb, 1, mj, False, mj == MJ - 1)
            pv_finish(po1, nb, 1, nc.scalar)
            po2 = vps.tile([128, CA], f32, name="pv", tag="pv")
            for mj in range(MJ):
                pv_mm(po2, ptb, 2, mj, mj == 0, mj == MJ - 1)
            pv_finish(po2, nb, 2, nc.scalar)
            po3 = vps.tile([128, CA], f32, name="pv", tag="pv")
            for mj in range(MJ):
                pv_mm(po3, ptb, 3, mj, mj == 0, mj == MJ - 1)
            ob = op.tile([128, CA], BF16, name="ob", tag="ob")
            nc.scalar.activation(ob[:, 0:C], po3[:, 0:C], Copy)
            nc.vector.tensor_copy(ob[:, C:C + 2].bitcast(F32),
                                  po3[:, C:C + 1])
            r = nb * FQ + 3 * 128
            nc.scalar.dma_start(out_d.ap()[r:r + 64, :], ob[0:64, :])
            nc.sync.dma_start(out_d.ap()[r + 64:r + 128, :], ob[64:128, :])

        # phase0/ST0 cascade, piece-major: each xq piece feeds its kk
        # segment, which immediately unblocks the matching ST0 chunks.
        pt0 = app.tile([128, MJ * FQ], BF16, name="pt", tag="pt")
        # warm-fill inside ST0 bridges the kk piece arrivals (the PE
        # must stay gap-free or the HAM re-throttles to 1.2 GHz, which
        # also halves the DMA rate). One insurance mm per early chunk
        # absorbs run-to-run DMA jitter at ~213 ns each when unneeded.
        for mj in range(8):
            st_range(0, pt0, mj, mj + 1)
            warm_mms(1)
        st_range(0, pt0, 8, MJ)
        pts = [pt0]
        for nb in range(1, NB):
            ptb = app.tile([128, MJ * FQ], BF16, name="pt", tag="pt")
            pts.append(ptb)
            st_range(nb, ptb, 0, MJ)
            pv_block(nb - 1, pts[nb - 1], False)
        pv_block(NB - 1, pts[NB - 1], True)


_CACHE = {}


def _build():
    if "nc" in _CACHE:
        return _CACHE["nc"]
    nc = bacc.Bacc("TRN2", target_bir_lowering=False, debug=False)
    kk_d = nc.dram_tensor("kk", [C, MHALF], F16, kind="ExternalInput")
    xq_d = nc.dram_tensor("xq", [C, N], F16, kind="ExternalInput")
    vt_d = nc.dram_tensor("vt", [128, MJ * CA], BF16, kind="ExternalInput")
    tb_d = nc.dram_tensor("tb", [128, MJ], F32, kind="ExternalInput")
    out_d = nc.dram_tensor("out", [N, CA], BF16, kind="ExternalOutput")
    with tile.TileContext(nc) as tc:
        _emit(nc, tc, kk_d, xq_d, vt_d, tb_d, out_d)
    nc.compile()
    _CACHE["nc"] = nc
    return nc


def _in_maps(img, label, z, wq, bq, wk, bk, wv, bv):
    bf16 = mybir.dt.np(BF16)
    x = np.concatenate(
        [np.asarray(img), np.asarray(label), np.asarray(z)], axis=1
    ).reshape(B, C, N).astype(np.float32)
    wq64 = np.asarray(wq, np.float64)
    wk64 = np.asarray(wk, np.float64)
    A = (wq64.T @ wk64).astype(np.float32)  # S = x^T A x + t
    u = (wk64.T @ np.asarray(bq, np.float64)).astype(np.float32)
    wvf = np.asarray(wv, np.float32)
    maps = []
    for b in range(B):
        kkb = A @ x[b]                      # [C, N]
        tbf = u @ x[b]                      # [N]
        vb = wvf @ x[b]                     # [C, N] (bv added on host)
        xq16 = np.ascontiguousarray(x[b].astype(np.float16))
        for h in range(2):
            sl = slice(h * MHALF, (h + 1) * MHALF)
            kk16 = np.ascontiguousarray(kkb[:, sl].astype(np.float16))
            tbv = np.ascontiguousarray(
                tbf[sl].reshape(MJ, 128).T.astype(np.float32)
            ) - np.float32(SHIFT)
            vt = np.ones((128, MJ, CA), np.float32)
            vt[:, :, 0:C] = vb[:, sl].reshape(C, MJ, 128).transpose(2, 1, 0)
            vt16 = np.ascontiguousarray(
                vt.reshape(128, MJ * CA).astype(bf16))
            maps.append({"kk": kk16, "xq": xq16, "vt": vt16, "tb": tbv})
    return maps


def kernel(img, label, z, wq, bq, wk, bk, wv, bv):
    nc = _build()
    maps = _in_maps(img, label, z, wq, bq, wk, bk, wv, bv)
    res = bass_utils.run_bass_kernel_spmd(nc, maps,
                                          core_ids=list(range(NCORES)))

    def _split(raw):
        # vals in bf16; Z arrives as raw fp32 bits spread over the last
        # two bf16 columns
        u = np.ascontiguousarray(raw).view(np.uint16)
        zz = (u[:, C].astype(np.uint32)
              | (u[:, C + 1].astype(np.uint32) << 16)).view(np.float32)
        return raw[:, 0:C].astype(np.float32), zz.reshape(-1, 1)

    out = np.empty((B, C, N), np.float32)
    bvf = np.asarray(bv, np.float32).reshape(1, C)
    for b in range(B):
        o0, z0 = _split(res.results[2 * b]["out"])
        o1, z1 = _split(res.results[2 * b + 1]["out"])
        o = o0 + o1
        zz = z0 + z1
        out[b] = ((o / zz) + bvf).T
    return out.reshape(B, C, H, W)


# revision 39
# speedup vs baseline: 1.0225x; 1.0069x over previous
"""CrossModalAttention Trainium2 kernel.

Full inputs -> full output. Internally: 8-way SPMD over (batch, key-half):
core = 2*b + h owns keys [h*2048, (h+1)*2048) of batch b and computes the
UNNORMALIZED attention output over those keys for ALL 4096 queries, plus
the per-query partition sum Z. The host sums the two partials per batch
and normalizes.

Math (per batch), with x = concat(img, label, z) [C=256, N=4096]:
  q = wq x + bq, k = wk x (bk dropped: constant-in-key terms cancel in
  softmax), v = wv x + bv
  S[n, m] = q_n . k_m = x_n^T A x_m + t_m       A = wq^T wk,  t = (wk^T bq)^T x
All three projections (kk = A x on the key side, t, vT) are computed on
the HOST: the device runs only the two N^2 C stages (scores + PV), in
fp16 operands (fp32 PSUM accumulation) so LDWEIGHTS gets fast-weight-load
and the input DMA bytes halve vs fp32.

Layouts (all per core, m = this core's 2048 keys):
  ST[m, n] via lhsT = kk chunk [128c, 128m], rhs = xq [128c, 512n]
  P = exp(ST + (t[m] - SHIFT))  -> bf16 pt
  out[n, c] = sum_m pt[m, n] * vT[m, c]; vT has ones columns appended so
  the same accumulation yields Z[n]. Raw (out|Z) goes to HBM; the host
  sums the two key-halves, divides by Z and adds bv.

Schedule notes:
- ~6 warmup matmuls on a zeroed fp16 tile keep the PE busy from the end
  of the framework preamble until the first kk/xq pieces land, so the
  HAM clock-gate sees one continuous busy window (2.4 GHz by ~10.7 us).
- Inputs ride four DMA queues (sync/gpsimd: kk halves + vT tail,
  scalar: xq, vector: vT head), pieces ordered by first-use.
- Score PSUM tiles are [128,512] (1 bank) with bufs=6: the exp (ACT) is
  slightly slower per tile than the 2 matmuls that fill it, and a deep
  rotation absorbs the drift without stalling the PE.
- Block interleave [ST0][ST1][PV0][ST2][PV1]...: PV(nb) starts a full
  score block after ST(nb), so exp(nb) is always done; pt is
  double-buffered. The last PV block interleaves its first two
  sub-blocks to cover the final exp tail.
- SHIFT=85 as in the proven baseline: scores lie in [-128, 132], exp in
  fp32 range with margin. Partial sums stay finite in fp32 (Z <= 2048*e^47).
"""

import numpy as np

import concourse.bacc as bacc
import concourse.mybir as mybir
import concourse.tile as tile
from concourse import bass_utils

B = 4
C = 256  # channels after concat
H = W = 64
N = H * W  # 4096 pixels
NCORES = 8
MHALF = N // 2  # 2048 keys per core
SHIFT = 85.0

F32 = mybir.dt.float32
F16 = mybir.dt.float16
BF16 = mybir.dt.bfloat16

FQ = 512  # query-block free dim
NB = N // FQ  # 8 query blocks per core (all queries)
MJ = MHALF // 128  # 16 key chunks of 128
CA = C + 2  # channels + ones col + pad (even free dim for the PE)


def _emit(nc, tc, kk_d, xq_d, vt_d, tb_d, out_d):
    f32 = F32
    mm = nc.tensor.matmul
    Exp = mybir.ActivationFunctionType.Exp
    Copy = mybir.ActivationFunctionType.Copy

    with tc.tile_pool(name="consts", bufs=1) as cp, \
         tc.tile_pool(name="xp", bufs=1) as xp, \
         tc.tile_pool(name="bigps", bufs=6, space="PSUM") as bigps, \
         tc.tile_pool(name="attn", bufs=2) as app, \
         tc.tile_pool(name="ob", bufs=2) as op, \
         tc.tile_pool(name="ob4", bufs=3) as op4, \
         tc.tile_pool(name="vps", bufs=2, space="PSUM") as vps:
        tb = cp.tile([128, MJ], f32, name="tb", tag="tb")
        warm = cp.tile([128, 512], F16, name="warm", tag="warm")
        # split memset: the first warmup LDWEIGHTS only needs cols 0:128
        nc.vector.memset(warm[:, 0:128], 0.0)
        nc.vector.memset(warm[:, 128:512], 0.0)

        kk = [xp.tile([128, MHALF], F16, name=f"kk{i}", tag=f"kk{i}")
              for i in range(2)]
        xq = [xp.tile([128, N], F16, name=f"xq{i}", tag=f"xq{i}")
              for i in range(2)]
        vT = xp.tile([128, MJ * CA], BF16, name="vT", tag="vT")

        # ---- DMAs: three queues (only sync/gpsimd/scalar can kick),
        # pieces ordered by first use. xq is host-rotated so this
        # core's 2048 keys are columns 0:2048 — the leading xq pieces
        # feed BOTH the kk projection (keys) and ST0-3 (queries). The
        # kk cascade consumes cols ascending, so xq streams in order.
        # AT (tiny) + tb lead on scalar; vT (needed from PV0, ~25 us)
        # rides scalar after them, keeping sync/gpsimd free for output
        # pieces from ~25 us on.
        nc.sync.dma_start(xq[0][:, 0:512], xq_d.ap()[0:128, 0:512])
        nc.sync.dma_start(kk[0][:, 0:256], kk_d.ap()[0:128, 0:256])
        nc.sync.dma_start(kk[0][:, 256:1024], kk_d.ap()[0:128, 256:1024])
        nc.sync.dma_start(xq[0][:, 512:1024], xq_d.ap()[0:128, 512:1024])
        nc.sync.dma_start(vT[:, 6 * CA:11 * CA], vt_d.ap()[:, 6 * CA:11 * CA])
        nc.sync.dma_start(xq[0][:, 1024:2048], xq_d.ap()[0:128, 1024:2048])
        nc.sync.dma_start(xq[0][:, 2048:4096], xq_d.ap()[0:128, 2048:4096])
        nc.gpsimd.dma_start(xq[1][:, 0:512], xq_d.ap()[128:256, 0:512])
        nc.gpsimd.dma_start(kk[1][:, 0:256], kk_d.ap()[128:256, 0:256])
        nc.gpsimd.dma_start(kk[1][:, 256:1024], kk_d.ap()[128:256, 256:1024])
        nc.gpsimd.dma_start(xq[1][:, 512:1024], xq_d.ap()[128:256, 512:1024])
        nc.gpsimd.dma_start(vT[:, 11 * CA:16 * CA],
                            vt_d.ap()[:, 11 * CA:16 * CA])
        nc.gpsimd.dma_start(xq[1][:, 1024:2048], xq_d.ap()[128:256, 1024:2048])
        nc.gpsimd.dma_start(xq[1][:, 2048:4096],
                            xq_d.ap()[128:256, 2048:4096])
        nc.scalar.dma_start(tb[:], tb_d.ap()[:, :])
        nc.scalar.dma_start(kk[0][:, 1024:2048], kk_d.ap()[0:128, 1024:2048])
        nc.scalar.dma_start(kk[1][:, 1024:2048], kk_d.ap()[128:256, 1024:2048])
        nc.scalar.dma_start(vT[:, 0:6 * CA], vt_d.ap()[:, 0:6 * CA])

        # ---- PE warmup (HAM un-throttle): keep the PE busy from the
        # end of the framework preamble until the first input pieces
        # land, so the clock-gate sees one continuous busy window.
        wps = bigps.tile([128, 512], f32, name="wps", tag="ps")

        def warm_mms(n):
            for _ in range(n):
                mm(wps[:], warm[:, 0:128], warm[:], start=True, stop=True)

        # Warmup bridges from the end of the framework preamble until
        # the DMA stream catches up (~12 us): one continuous PE-busy
        # window so the HAM clock-gate reaches 2.4 GHz at ~10.7 us
        # (which also doubles the DMA rate).
        warm_mms(10)

        # ---- attention ----
        def st_range(nb, ptb, mja, mjb):
            for mj in range(mja, mjb):
                ps = bigps.tile([128, 512], f32, name="st", tag="ps")
                for ci in range(2):
                    mm(ps[:], kk[ci][:, mj * 128:(mj + 1) * 128],
                       xq[ci][:, nb * FQ:(nb + 1) * FQ],
                       start=ci == 0, stop=ci == 1)
                nc.scalar.activation(
                    ptb[:, mj * FQ:(mj + 1) * FQ], ps[:], Exp,
                    bias=tb[:, mj:mj + 1])

        def pv_mm(po, ptb, ns, mj, start, stop):
            o = mj * FQ + ns * 128
            mm(po[:], ptb[:, o:o + 128], vT[:, mj * CA:(mj + 1) * CA],
               start=start, stop=stop)

        # Output rides as bf16 (the DMA-ring write path is the scarce
        # resource). The fp32 Z is bit-split across the last two bf16
        # columns (exact — the host reassembles the fp32 bits).
        def pv_finish(po, nb, ns, eng=None):
            ob = op.tile([128, CA], BF16, name="ob", tag="ob")
            nc.vector.tensor_copy(ob[:, 0:C], po[:, 0:C])
            nc.vector.tensor_copy(ob[:, C:C + 2].bitcast(F32),
                                  po[:, C:C + 1])
            r = nb * FQ + ns * 128
            if eng is None:
                # scalar's input ring drains by ~16 us (sync/gpsimd
                # still carry the xq tails) — give it half the output.
                eng = (nc.scalar, nc.gpsimd, nc.scalar, nc.sync)[ns]
            eng.dma_start(out_d.ap()[r:r + 128, :], ob[:])

        def pv_block(nb, ptb, last):
            if not last:
                # one strided DMA ships the whole 512-query block: fewer
                # kicks + semaphores (shorter epilogue, less engine time)
                ob4 = op4.tile([128, 4 * CA], BF16, name="ob4", tag="ob4")
                for ns in range(4):
                    po = vps.tile([128, CA], f32, name="pv", tag="pv")
                    for mj in range(MJ):
                        pv_mm(po, ptb, ns, mj, mj == 0, mj == MJ - 1)
                    o = ns * CA
                    nc.vector.tensor_copy(ob4[:, o:o + C], po[:, 0:C])
                    nc.vector.tensor_copy(
                        ob4[:, o + C:o + C + 2].bitcast(F32), po[:, C:C + 1])
                r = nb * FQ
                eng = (nc.scalar, nc.gpsimd, nc.sync)[nb % 3]
                eng.dma_start(
                    out_d.ap()[r:r + FQ, :].rearrange("(b p) c -> p b c",
                                                      p=128),
                    ob4[:].rearrange("p (b c) -> p b c", c=CA))
                return
            # last block: interleave the first two sub-blocks so the
            # accumulation never waits on the trailing exp chunks.
            po0 = vps.tile([128, CA], f32, name="pv0", tag="pv")
            for mj in range(12):
                pv_mm(po0, ptb, 0, mj, mj == 0, False)
            po1 = vps.tile([128, CA], f32, name="pv1", tag="pv")
            for mj in range(8):
                pv_mm(po1, ptb, 1, mj, mj == 0, False)
            # final-block tail: all DMAs ride the scalar queue (its ring
            # is empty by now — the gpsimd/sync rings still carry earlier
            # blocks), and the last sub-block's Z hi/lo chain runs on
            # scalar in parallel with the bulk copy on vector.
            for mj in range(12, MJ):
                pv_mm(po0, ptb, 0, mj, False, mj == MJ - 1)
            pv_finish(po0, nb, 0, nc.scalar)
            for mj in range(8, MJ):
                pv_mm(po1, ptb, 1, mj, False, mj == MJ - 1)
            pv_finish(po1, nb, 1, nc.scalar)
            po2 = vps.tile([128, CA], f32, name="pv", tag="pv")
            for mj in range(MJ):
                pv_mm(po2, ptb, 2, mj, mj == 0, mj == MJ - 1)
            pv_finish(po2, nb, 2, nc.scalar)
            po3 = vps.tile([128, CA], f32, name="pv", tag="pv")
            for mj in range(MJ):
                pv_mm(po3, ptb, 3, mj, mj == 0, mj == MJ - 1)
            ob = op.tile([128, CA], BF16, name="ob", tag="ob")
            nc.scalar.activation(ob[:, 0:C], po3[:, 0:C], Copy)
            nc.vector.tensor_copy(ob[:, C:C + 2].bitcast(F32),
                                  po3[:, C:C + 1])
            r = nb * FQ + 3 * 128
            nc.scalar.dma_start(out_d.ap()[r:r + 64, :], ob[0:64, :])
            nc.sync.dma_start(out_d.ap()[r + 64:r + 128, :], ob[64:128, :])

        # phase0/ST0 cascade, piece-major: each xq piece feeds its kk
        # segment, which immediately unblocks the matching ST0 chunks.
        pt0 = app.tile([128, MJ * FQ], BF16, name="pt", tag="pt")
        # warm-fill inside ST0 bridges the kk piece arrivals (the PE
        # must stay gap-free or the HAM re-throttles to 1.2 GHz, which
        # also halves the DMA rate). One insurance mm per early chunk
        # absorbs run-to-run DMA jitter at ~213 ns each when unneeded.
        for mj in range(8):
            st_range(0, pt0, mj, mj + 1)
            warm_mms(1)
        st_range(0, pt0, 8, MJ)
        pts = [pt0]
        for nb in range(1, NB):
            ptb = app.tile([128, MJ * FQ], BF16, name="pt", tag="pt")
            pts.append(ptb)
            st_range(nb, ptb, 0, MJ)
            pv_block(nb - 1, pts[nb - 1], False)
        pv_block(NB - 1, pts[NB - 1], True)


_CACHE = {}


def _build():
    if "nc" in _CACHE:
        return _CACHE["nc"]
    nc = bacc.Bacc("TRN2", target_bir_lowering=False, debug=False)
    kk_d = nc.dram_tensor("kk", [C, MHALF], F16, kind="ExternalInput")
    xq_d = nc.dram_tensor("xq", [C, N], F16, kind="ExternalInput")
    vt_d = nc.dram_tensor("vt", [128, MJ * CA], BF16, kind="ExternalInput")
    tb_d = nc.dram_tensor("tb", [128, MJ], F32, kind="ExternalInput")
    out_d = nc.dram_tensor("out", [N, CA], BF16, kind="ExternalOutput")
    with tile.TileContext(nc) as tc:
        _emit(nc, tc, kk_d, xq_d, vt_d, tb_d, out_d)
    nc.compile()
    _CACHE["nc"] = nc
    return nc


def _in_maps(img, label, z, wq, bq, wk, bk, wv, bv):
    bf16 = mybir.dt.np(BF16)
    x = np.concatenate(
        [np.asarray(img), np.asarray(label), np.asarray(z)], axis=1
    ).reshape(B, C, N).astype(np.float32)
    wq64 = np.asarray(wq, np.float64)
    wk64 = np.asarray(wk, np.float64)
    A = (wq64.T @ wk64).astype(np.float32)  # S = x^T A x + t
    u = (wk64.T @ np.asarray(bq, np.float64)).astype(np.float32)
    wvf = np.asarray(wv, np.float32)
    maps = []
    for b in range(B):
        kkb = A @ x[b]                      # [C, N]
        tbf = u @ x[b]                      # [N]
        vb = wvf @ x[b]                     # [C, N] (bv added on host)
        xq16 = np.ascontiguousarray(x[b].astype(np.float16))
        for h in range(2):
            sl = slice(h * MHALF, (h + 1) * MHALF)
            kk16 = np.ascontiguousarray(kkb[:, sl].astype(np.float16))
            tbv = np.ascontiguousarray(
                tbf[sl].reshape(MJ, 128).T.astype(np.float32)
            ) - np.float32(SHIFT)
            vt = np.ones((128, MJ, CA), np.float32)
            vt[:, :, 0:C] = vb[:, sl].reshape(C, MJ, 128).transpose(2, 1, 0)
            vt16 = np.ascontiguousarray(
                vt.reshape(128, MJ * CA).astype(bf16))
            maps.append({"kk": kk16, "xq": xq16, "vt": vt16, "tb": tbv})
    return maps


def kernel(img, label, z, wq, bq, wk, bk, wv, bv):
    nc = _build()
    maps = _in_maps(img, label, z, wq, bq, wk, bk, wv, bv)
    res = bass_utils.run_bass_kernel_spmd(nc, maps,
                                          core_ids=list(range(NCORES)))

    def _split(raw):
        # vals in bf16; Z arrives as raw fp32 bits spread over the last
        # two bf16 columns
        u = np.ascontiguousarray(raw).view(np.uint16)
        zz = (u[:, C].astype(np.uint32)
              | (u[:, C + 1].astype(np.uint32) << 16)).view(np.float32)
        return raw[:, 0:C].astype(np.float32), zz.reshape(-1, 1)

    out = np.empty((B, C, N), np.float32)
    bvf = np.asarray(bv, np.float32).reshape(1, C)
    for b in range(B):
        o0, z0 = _split(res.results[2 * b]["out"])
        o1, z1 = _split(res.results[2 * b + 1]["out"])
        o = o0 + o1
        zz = z0 + z1
        out[b] = ((o / zz) + bvf).T
    return out.reshape(B, C, H, W)


# revision 41
# speedup vs baseline: 1.0242x; 1.0017x over previous
"""CrossModalAttention Trainium2 kernel.

Full inputs -> full output. Internally: 8-way SPMD over (batch, key-half):
core = 2*b + h owns keys [h*2048, (h+1)*2048) of batch b and computes the
UNNORMALIZED attention output over those keys for ALL 4096 queries, plus
the per-query partition sum Z. The host sums the two partials per batch
and normalizes.

Math (per batch), with x = concat(img, label, z) [C=256, N=4096]:
  q = wq x + bq, k = wk x (bk dropped: constant-in-key terms cancel in
  softmax), v = wv x + bv
  S[n, m] = q_n . k_m = x_n^T A x_m + t_m       A = wq^T wk,  t = (wk^T bq)^T x
All three projections (kk = A x on the key side, t, vT) are computed on
the HOST: the device runs only the two N^2 C stages (scores + PV), in
fp16 operands (fp32 PSUM accumulation) so LDWEIGHTS gets fast-weight-load
and the input DMA bytes halve vs fp32.

Layouts (all per core, m = this core's 2048 keys):
  ST[m, n] via lhsT = kk chunk [128c, 128m], rhs = xq [128c, 512n]
  P = exp(ST + (t[m] - SHIFT))  -> bf16 pt
  out[n, c] = sum_m pt[m, n] * vT[m, c]; vT has ones columns appended so
  the same accumulation yields Z[n]. Raw (out|Z) goes to HBM; the host
  sums the two key-halves, divides by Z and adds bv.

Schedule notes:
- ~6 warmup matmuls on a zeroed fp16 tile keep the PE busy from the end
  of the framework preamble until the first kk/xq pieces land, so the
  HAM clock-gate sees one continuous busy window (2.4 GHz by ~10.7 us).
- Inputs ride four DMA queues (sync/gpsimd: kk halves + vT tail,
  scalar: xq, vector: vT head), pieces ordered by first-use.
- Score PSUM tiles are [128,512] (1 bank) with bufs=6: the exp (ACT) is
  slightly slower per tile than the 2 matmuls that fill it, and a deep
  rotation absorbs the drift without stalling the PE.
- Block interleave [ST0][ST1][PV0][ST2][PV1]...: PV(nb) starts a full
  score block after ST(nb), so exp(nb) is always done; pt is
  double-buffered. The last PV block interleaves its first two
  sub-blocks to cover the final exp tail.
- SHIFT=85 as in the proven baseline: scores lie in [-128, 132], exp in
  fp32 range with margin. Partial sums stay finite in fp32 (Z <= 2048*e^47).
"""

import numpy as np

import concourse.bacc as bacc
import concourse.mybir as mybir
import concourse.tile as tile
from concourse import bass_utils

B = 4
C = 256  # channels after concat
H = W = 64
N = H * W  # 4096 pixels
NCORES = 8
MHALF = N // 2  # 2048 keys per core
SHIFT = 85.0

F32 = mybir.dt.float32
F16 = mybir.dt.float16
BF16 = mybir.dt.bfloat16

FQ = 512  # query-block free dim
NB = N // FQ  # 8 query blocks per core (all queries)
MJ = MHALF // 128  # 16 key chunks of 128
CA = C + 2  # channels + ones col + pad (even free dim for the PE)


def _emit(nc, tc, kk_d, xq_d, vt_d, tb_d, out_d):
    f32 = F32
    mm = nc.tensor.matmul
    Exp = mybir.ActivationFunctionType.Exp
    Copy = mybir.ActivationFunctionType.Copy

    with tc.tile_pool(name="consts", bufs=1) as cp, \
         tc.tile_pool(name="xp", bufs=1) as xp, \
         tc.tile_pool(name="bigps", bufs=6, space="PSUM") as bigps, \
         tc.tile_pool(name="attn", bufs=2) as app, \
         tc.tile_pool(name="ob", bufs=2) as op, \
         tc.tile_pool(name="ob4", bufs=3) as op4, \
         tc.tile_pool(name="vps", bufs=2, space="PSUM") as vps:
        tb = cp.tile([128, MJ], f32, name="tb", tag="tb")
        warm = cp.tile([128, 512], F16, name="warm", tag="warm")
        # split memset: the first warmup LDWEIGHTS only needs cols 0:128
        nc.vector.memset(warm[:, 0:128], 0.0)
        nc.vector.memset(warm[:, 128:512], 0.0)

        kk = [xp.tile([128, MHALF], F16, name=f"kk{i}", tag=f"kk{i}")
              for i in range(2)]
        xq = [xp.tile([128, N], F16, name=f"xq{i}", tag=f"xq{i}")
              for i in range(2)]
        vT = xp.tile([128, MJ * CA], BF16, name="vT", tag="vT")

        # ---- DMAs: three queues (only sync/gpsimd/scalar can kick),
        # pieces ordered by first use. xq is host-rotated so this
        # core's 2048 keys are columns 0:2048 — the leading xq pieces
        # feed BOTH the kk projection (keys) and ST0-3 (queries). The
        # kk cascade consumes cols ascending, so xq streams in order.
        # AT (tiny) + tb lead on scalar; vT (needed from PV0, ~25 us)
        # rides scalar after them, keeping sync/gpsimd free for output
        # pieces from ~25 us on.
        nc.sync.dma_start(xq[0][:, 0:512], xq_d.ap()[0:128, 0:512])
        nc.sync.dma_start(kk[0][:, 0:256], kk_d.ap()[0:128, 0:256])
        nc.sync.dma_start(kk[0][:, 256:1024], kk_d.ap()[0:128, 256:1024])
        nc.sync.dma_start(xq[0][:, 512:1024], xq_d.ap()[0:128, 512:1024])
        nc.sync.dma_start(vT[:, 6 * CA:11 * CA], vt_d.ap()[:, 6 * CA:11 * CA])
        nc.sync.dma_start(xq[0][:, 1024:2048], xq_d.ap()[0:128, 1024:2048])
        nc.sync.dma_start(xq[0][:, 2048:4096], xq_d.ap()[0:128, 2048:4096])
        nc.gpsimd.dma_start(xq[1][:, 0:512], xq_d.ap()[128:256, 0:512])
        nc.gpsimd.dma_start(kk[1][:, 0:256], kk_d.ap()[128:256, 0:256])
        nc.gpsimd.dma_start(kk[1][:, 256:1024], kk_d.ap()[128:256, 256:1024])
        nc.gpsimd.dma_start(xq[1][:, 512:1024], xq_d.ap()[128:256, 512:1024])
        nc.gpsimd.dma_start(vT[:, 11 * CA:16 * CA],
                            vt_d.ap()[:, 11 * CA:16 * CA])
        nc.gpsimd.dma_start(xq[1][:, 1024:2048], xq_d.ap()[128:256, 1024:2048])
        nc.gpsimd.dma_start(xq[1][:, 2048:4096],
                            xq_d.ap()[128:256, 2048:4096])
        nc.scalar.dma_start(tb[:], tb_d.ap()[:, :])
        nc.scalar.dma_start(kk[0][:, 1024:2048], kk_d.ap()[0:128, 1024:2048])
        nc.scalar.dma_start(kk[1][:, 1024:2048], kk_d.ap()[128:256, 1024:2048])
        nc.scalar.dma_start(vT[:, 0:6 * CA], vt_d.ap()[:, 0:6 * CA])

        # ---- PE warmup (HAM un-throttle): keep the PE busy from the
        # end of the framework preamble until the first input pieces
        # land, so the clock-gate sees one continuous busy window.
        wps = bigps.tile([128, 512], f32, name="wps", tag="ps")

        def warm_mms(n):
            for _ in range(n):
                mm(wps[:], warm[:, 0:128], warm[:], start=True, stop=True)

        # Warmup bridges from the end of the framework preamble until
        # the DMA stream catches up (~12 us): one continuous PE-busy
        # window so the HAM clock-gate reaches 2.4 GHz at ~10.7 us
        # (which also doubles the DMA rate).
        warm_mms(10)

        # ---- attention ----
        def st_range(nb, ptb, mja, mjb):
            for mj in range(mja, mjb):
                ps = bigps.tile([128, 512], f32, name="st", tag="ps")
                for ci in range(2):
                    mm(ps[:], kk[ci][:, mj * 128:(mj + 1) * 128],
                       xq[ci][:, nb * FQ:(nb + 1) * FQ],
                       start=ci == 0, stop=ci == 1)
                nc.scalar.activation(
                    ptb[:, mj * FQ:(mj + 1) * FQ], ps[:], Exp,
                    bias=tb[:, mj:mj + 1])

        def pv_mm(po, ptb, ns, mj, start, stop):
            o = mj * FQ + ns * 128
            mm(po[:], ptb[:, o:o + 128], vT[:, mj * CA:(mj + 1) * CA],
               start=start, stop=stop)

        # Output rides as bf16 (the DMA-ring write path is the scarce
        # resource). The fp32 Z is bit-split across the last two bf16
        # columns (exact — the host reassembles the fp32 bits).
        def pv_finish(po, nb, ns, eng=None):
            ob = op.tile([128, CA], BF16, name="ob", tag="ob")
            nc.vector.tensor_copy(ob[:, 0:C], po[:, 0:C])
            nc.vector.tensor_copy(ob[:, C:C + 2].bitcast(F32),
                                  po[:, C:C + 1])
            r = nb * FQ + ns * 128
            if eng is None:
                # scalar's input ring drains by ~16 us (sync/gpsimd
                # still carry the xq tails) — give it half the output.
                eng = (nc.scalar, nc.gpsimd, nc.scalar, nc.sync)[ns]
            eng.dma_start(out_d.ap()[r:r + 128, :], ob[:])

        def pv_block(nb, ptb, last):
            if not last:
                # one strided DMA ships the whole 512-query block: fewer
                # kicks + semaphores (shorter epilogue, less engine time)
                ob4 = op4.tile([128, 4 * CA], BF16, name="ob4", tag="ob4")
                for ns in range(4):
                    po = vps.tile([128, CA], f32, name="pv", tag="pv")
                    for mj in range(MJ):
                        pv_mm(po, ptb, ns, mj, mj == 0, mj == MJ - 1)
                    o = ns * CA
                    nc.vector.tensor_copy(ob4[:, o:o + C], po[:, 0:C])
                    nc.vector.tensor_copy(
                        ob4[:, o + C:o + C + 2].bitcast(F32), po[:, C:C + 1])
                r = nb * FQ
                eng = (nc.scalar, nc.gpsimd, nc.sync)[nb % 3]
                eng.dma_start(
                    out_d.ap()[r:r + FQ, :].rearrange("(b p) c -> p b c",
                                                      p=128),
                    ob4[:].rearrange("p (b c) -> p b c", c=CA))
                return
            # last block: interleave the first two sub-blocks so the
            # accumulation never waits on the trailing exp chunks.
            po0 = vps.tile([128, CA], f32, name="pv0", tag="pv")
            for mj in range(12):
                pv_mm(po0, ptb, 0, mj, mj == 0, False)
            po1 = vps.tile([128, CA], f32, name="pv1", tag="pv")
            for mj in range(8):
                pv_mm(po1, ptb, 1, mj, mj == 0, False)
            # final-block tail: all DMAs ride the scalar queue (its ring
            # is empty by now — the gpsimd/sync rings still carry earlier
            # blocks), and the last sub-block's Z hi/lo chain runs on
            # scalar in parallel with the bulk copy on vector.
            for mj in range(12, MJ):
                pv_mm(po0, ptb, 0, mj, False, mj == MJ - 1)
            pv_finish(po0, nb, 0, nc.scalar)
            for mj in range(8, MJ):
                pv_mm(po1, ptb, 1, mj, False, mj == MJ - 1)
            pv_finish(po1, nb, 1, nc.scalar)
            po2 = vps.tile([128, CA], f32, name="pv", tag="pv")
            for mj in range(MJ):
                pv_mm(po2, ptb, 2, mj, mj == 0, mj == MJ - 1)
            pv_finish(po2, nb, 2, nc.scalar)
            po3 = vps.tile([128, CA], f32, name="pv", tag="pv")
            for mj in range(MJ):
                pv_mm(po3, ptb, 3, mj, mj == 0, mj == MJ - 1)
            ob = op.tile([128, CA], BF16, name="ob", tag="ob")
            # bulk copy on vector (~160 ns vs ~470 for scalar ACT); the
            # tiny Z bitcast rides scalar in parallel.
            nc.vector.tensor_copy(ob[:, 0:C], po3[:, 0:C])
            nc.scalar.activation(ob[:, C:C + 2].bitcast(F32),
                                 po3[:, C:C + 1], Copy)
            r = nb * FQ + 3 * 128
            nc.scalar.dma_start(out_d.ap()[r:r + 64, :], ob[0:64, :])
            nc.sync.dma_start(out_d.ap()[r + 64:r + 128, :], ob[64:128, :])

        # phase0/ST0 cascade, piece-major: each xq piece feeds its kk
        # segment, which immediately unblocks the matching ST0 chunks.
        pt0 = app.tile([128, MJ * FQ], BF16, name="pt", tag="pt")
        # warm-fill inside ST0 bridges the kk piece arrivals (the PE
        # must stay gap-free or the HAM re-throttles to 1.2 GHz, which
        # also halves the DMA rate). One insurance mm per early chunk
        # absorbs run-to-run DMA jitter at ~213 ns each when unneeded.
        for mj in range(6):
            st_range(0, pt0, mj, mj + 1)
            warm_mms(1)
        st_range(0, pt0, 6, MJ)
        pts = [pt0]
        for nb in range(1, NB):
            ptb = app.tile([128, MJ * FQ], BF16, name="pt", tag="pt")
            pts.append(ptb)
            st_range(nb, ptb, 0, MJ)
            pv_block(nb - 1, pts[nb - 1], False)
        pv_block(NB - 1, pts[NB - 1], True)


_CACHE = {}


def _build():
    if "nc" in _CACHE:
        return _CACHE["nc"]
    nc = bacc.Bacc("TRN2", target_bir_lowering=False, debug=False)
    kk_d = nc.dram_tensor("kk", [C, MHALF], F16, kind="ExternalInput")
    xq_d = nc.dram_tensor("xq", [C, N], F16, kind="ExternalInput")
    vt_d = nc.dram_tensor("vt", [128, MJ * CA], BF16, kind="ExternalInput")
    tb_d = nc.dram_tensor("tb", [128, MJ], F32, kind="ExternalInput")
    out_d = nc.dram_tensor("out", [N, CA], BF16, kind="ExternalOutput")
    with tile.TileContext(nc) as tc:
        _emit(nc, tc, kk_d, xq_d, vt_d, tb_d, out_d)
    nc.compile()
    _CACHE["nc"] = nc
    return nc


def _in_maps(img, label, z, wq, bq, wk, bk, wv, bv):
    bf16 = mybir.dt.np(BF16)
    x = np.concatenate(
        [np.asarray(img), np.asarray(label), np.asarray(z)], axis=1
    ).reshape(B, C, N).astype(np.float32)
    wq64 = np.asarray(wq, np.float64)
    wk64 = np.asarray(wk, np.float64)
    A = (wq64.T @ wk64).astype(np.float32)  # S = x^T A x + t
    u = (wk64.T @ np.asarray(bq, np.float64)).astype(np.float32)
    wvf = np.asarray(wv, np.float32)
    maps = []
    for b in range(B):
        kkb = A @ x[b]                      # [C, N]
        tbf = u @ x[b]                      # [N]
        vb = wvf @ x[b]                     # [C, N] (bv added on host)
        xq16 = np.ascontiguousarray(x[b].astype(np.float16))
        for h in range(2):
            sl = slice(h * MHALF, (h + 1) * MHALF)
            kk16 = np.ascontiguousarray(kkb[:, sl].astype(np.float16))
            tbv = np.ascontiguousarray(
                tbf[sl].reshape(MJ, 128).T.astype(np.float32)
            ) - np.float32(SHIFT)
            vt = np.ones((128, MJ, CA), np.float32)
            vt[:, :, 0:C] = vb[:, sl].reshape(C, MJ, 128).transpose(2, 1, 0)
            vt16 = np.ascontiguousarray(
                vt.reshape(128, MJ * CA).astype(bf16))
            maps.append({"kk": kk16, "xq": xq16, "vt": vt16, "tb": tbv})
    return maps


def kernel(img, label, z, wq, bq, wk, bk, wv, bv):
    nc = _build()
    maps = _in_maps(img, label, z, wq, bq, wk, bk, wv, bv)
    res = bass_utils.run_bass_kernel_spmd(nc, maps,
                                          core_ids=list(range(NCORES)))

    def _split(raw):
        # vals in bf16; Z arrives as raw fp32 bits spread over the last
        # two bf16 columns
        u = np.ascontiguousarray(raw).view(np.uint16)
        zz = (u[:, C].astype(np.uint32)
              | (u[:, C + 1].astype(np.uint32) << 16)).view(np.float32)
        return raw[:, 0:C].astype(np.float32), zz.reshape(-1, 1)

    out = np.empty((B, C, N), np.float32)
    bvf = np.asarray(bv, np.float32).reshape(1, C)
    for b in range(B):
        o0, z0 = _split(res.results[2 * b]["out"])
        o1, z1 = _split(res.results[2 * b + 1]["out"])
        o = o0 + o1
        zz = z0 + z1
        out[b] = ((o / zz) + bvf).T
    return out.reshape(B, C, H, W)
